# revision 35
# baseline (speedup 1.0000x reference)
"""Trainium2 Bass kernel for nn_AttnGate (block-sparse attention gate).

Computation (per batch b, kv-head kh):
    qp  = einsum('s(gd),(gd)o->so', q[b,:,4kh:4kh+4,:], wq[kh])       # [S, GH]
    qpR = rope(qp, cos_q[b], sin_q[b])
    kc  = [mean, max] pooling of k[b,:,kh,:] over 64-key blocks        # [NB, 2D]
    kp  = kc @ wk[kh];  kpR = rope(kp, cos_k[b], sin_k[b])             # [NB, GH]
    out = softmax(qpR @ kpR.T / sqrt(GH) + mask[b])                    # [S, NB]

Sharding: 16 (b, kh) units over 8 cores; core c handles b = c//4 and
kh in {2*(c%4), 2*(c%4)+1}.  No collectives; host slices/gathers.

Device/host split: the device does the dominant 99.8% of FLOPs — the
q projection (1.07 GFLOP/unit), the q-side rope multiply, the QK score
matmul and the exp.  The tiny K path (block pooling + [NB,2D]@[2D,GH]
projection + rope, ~0.2% of FLOPs) runs on the host, which shrinks the
device input stream by the whole 4.2 MB k tensor; only the finished
kpr/kq2 tables (131 KB) ship.  As in the baseline, the additive mask
and the softmax normalization are applied on the host (softmax(x+m) =
e^x e^m / sum — identical math), so the device ships unnormalized
e = exp(scores) in bf16 and the mask never moves to the device.

Per-core HBM traffic: q 16.8 MB fp16 + rope tables 4.2 MB fp16 +
out ~2.2 MB bf16 (causal) + residents ~0.4 MB ~= 23.6 MB, streamed at
~24-26 GB/s/engine x 16 DMA engines (~410 GB/s sustained).  The kernel
is paced by this stream; compute per tile (PE 1.3 us, DVE 1.2 us,
scalar 0.7 us) sits under the ~1.7 us/tile DMA budget, so tiles drain
as they land.  DMA engines service descriptors in global enqueue
order, so input transfers trigger in consumption order with a ~7-tile
lead (a larger lead or out-of-order / front-loaded transfers displace
later-needed bytes and stall the pipeline -- measured, not
theoretical).  The first few transfers issue on the three
trigger-capable queues (sync, scalar, gpsimd) in parallel to beat the
~0.64 us/trigger single-queue rate during the ramp; the rest stays on
the sync queue.

Score matmul orientation: kpr/kq2 are the PE *weights* ([GH, W]) and
the roped q-side streams its full 512-column s-tile, producing the
score tile transposed ([W, 512] in PSUM).  This needs only 2 matmul
instructions per tile (vs 8 short W-moving ones), avoiding the ~150 ns
per-instruction PE overhead that dominated the old orientation; the
host gather untransposes.  RoPE rotate-half is folded into the QK
matmul: attn[n,s] = sum_h kpr[h,n]*(qp*cos)[h,s] + kq2[h,n]*(qp*tq)[h,s]
with tq = swap_halves(sin_q) (upper half negated) and kq2 =
swap_halves(kpR), both built on host.

When the mask is the canonical block-causal pattern, a "causal" NEFF
ships only the W = 8t+8 visible score rows for s-tile t (PE time is
W-independent in this orientation; only output bytes shrink).  Any
other mask falls back to a generic full-width NEFF.

Host-side layout prep (numpy only):
  - q pre-transposed to contraction-major tiles [D, G, ST] fp16
    (4 KB contiguous per partition row).
  - 1/sqrt(GH) folded into wq.
  - rope tables packed [GH, 2, S] fp16 = {cos_q, tq}, quartered.
"""

import math
import os
import sys

import numpy as np

for _p in ("/opt/trn_rl_repo", "/root/.axon_site/_ro/trn_rl_repo"):
    if os.path.isdir(_p) and _p not in sys.path:
        sys.path.append(_p)

import ml_dtypes  # noqa: E402,F401

import concourse.bass as bass  # noqa: E402,F401
import concourse.bacc as bacc  # noqa: E402
import concourse.mybir as mybir  # noqa: E402
from concourse.bass_utils import run_bass_kernel_spmd  # noqa: E402
from concourse.tile import TileContext  # noqa: E402

# Problem dims (hardcoded per spec).
B, S, HQ, HK, D, GH = 2, 8192, 32, 8, 128, 128
BLK = 64
NB = S // BLK          # 128 key blocks
G = HQ // HK           # 4
GD = G * D             # 512 contraction for the q projection
H = GH // 2            # rotate-half split
ST = 512               # s-tile (matmul moving-dim = PSUM bank)
NT = S // ST           # 16 s-tiles per unit
TPC = 4                # s-tiles per q DMA chunk
NCH = NT // TPC        # 4 q chunks per unit
N_CORES = 8
UNITS = 2              # (b, kh) units per core

F32 = mybir.dt.float32
BF16 = mybir.dt.bfloat16
FP16 = mybir.dt.float16
FP = mybir.dt.np  # dt -> numpy dtype


def _np_dt(dt):
    return np.dtype(FP(dt))


def _width(t, causal):
    """Score rows computed for s-tile `t` (all unless causal)."""
    return 8 * t + 8 if causal else NB


def build_bass(causal):
    """Build the single-core SPMD Bass program (same NEFF on all 8 cores)."""
    nc = bacc.Bacc("TRN2")

    qT = nc.declare_dram_parameter("qT", [UNITS, NT, D, G, ST], FP16,
                                   isOutput=False)
    # packed small residents: wq [D, UNITS*G*GH] fp16, kpq [GH, UNITS, 2, NB]
    wqp = nc.declare_dram_parameter("wqp", [D, UNITS * G * GH], FP16,
                                    isOutput=False)
    kpq = nc.declare_dram_parameter("kpq", [GH, UNITS, 2, NB], FP16,
                                    isOutput=False)
    # q-side rope tables {cos, tq}, streamed in quarters
    ct = nc.declare_dram_parameter("ct", [GH, 2, S], FP16, isOutput=False)
    out = nc.declare_dram_parameter("out", [UNITS, NT, NB, ST], BF16,
                                    isOutput=True)

    NQUART = 4
    SQ = S // NQUART  # 2048 cols per rope-table quarter

    seq = [(j, t) for j in range(UNITS) for t in range(NT)]

    with TileContext(nc) as tc:
        with (
            tc.tile_pool(name="singles", bufs=1) as singles,
            tc.tile_pool(name="qin", bufs=16) as qin,
            tc.tile_pool(name="qps", bufs=4, space="PSUM") as qps,
            tc.tile_pool(name="atps", bufs=4, space="PSUM") as atps,
            tc.tile_pool(name="absb", bufs=6) as absb,
            tc.tile_pool(name="osb", bufs=6) as osb,
        ):
            # ---- resident + streamed DMAs, ordered so the first qproj is
            # gated on only 0.76 MB (wq unit-0 half + q tile 0) and the first
            # score chain on ~1.2 MB, instead of the whole resident set ----
            # The sync queue can only issue one DMA trigger per ~0.64 us, so
            # a single-queue start leaves the 16 DMA engines starved for the
            # first ~4 us.  The first five transfers therefore trigger on
            # the three trigger-capable queues (sync, scalar, gpsimd) in
            # parallel; the rest of the stream stays on sync in order.
            wq_sb = {}
            for j in range(UNITS):
                wq_sb[j] = singles.tile([D, G * GH], FP16, name=f"wq_sb{j}",
                                        tag=f"wq{j}")
            nc.sync.dma_start(out=wq_sb[0], in_=wqp[:, :G * GH])

            q_tiles = {}

            def load_q(j, t, eng=None):
                if (j, t) not in q_tiles:
                    tl = qin.tile([D, G, ST], FP16, name=f"q_{j}_{t}", tag="q")
                    (eng or nc.sync).dma_start(out=tl, in_=qT[j, t])
                    q_tiles[j, t] = tl

            load_q(0, 0, eng=nc.scalar)

            # ct quarter 0 ships split (tile-0 slice + tiles 1-3) so rope(0)
            # is not gated behind the full 1 MB quarter during the DMA ramp
            ct0a_sb = singles.tile([GH, 2, ST], FP16, name="ct0a_sb",
                                   tag="ct0a")
            nc.gpsimd.dma_start(out=ct0a_sb, in_=ct[:, :, :ST])
            kpq_sb = singles.tile([GH, UNITS, 2, NB], FP16, name="kpq_sb",
                                  tag="kpq")
            nc.sync.dma_start(out=kpq_sb, in_=kpq[:, :, :, :])
            load_q(0, 1, eng=nc.scalar)

            ct_sb = {}

            def load_quarter(qi):
                if 0 < qi < NQUART and qi not in ct_sb:
                    tl = singles.tile([GH, 2, SQ], FP16, name=f"ct_sb_{qi}",
                                      tag=f"ctq{qi}")
                    nc.sync.dma_start(out=tl,
                                      in_=ct[:, :, qi * SQ:(qi + 1) * SQ])
                    ct_sb[qi] = tl

            load_q(0, 2)
            ct0b_sb = singles.tile([GH, 2, SQ - ST], FP16, name="ct0b_sb",
                                   tag="ct0b")
            nc.sync.dma_start(out=ct0b_sb, in_=ct[:, :, ST:SQ])
            load_q(0, 3)
            nc.sync.dma_start(out=wq_sb[1], in_=wqp[:, G * GH:])
            load_quarter(1)

            # ---- compute building blocks ----
            qp_ps = {}

            def qproj(j, t):
                ps = qps.tile([GH, ST], F32, name=f"qp_{j}_{t}", tag="qp")
                qt = q_tiles[j, t]
                for c in range(G):
                    nc.tensor.matmul(
                        ps, wq_sb[j][:, c * GH:(c + 1) * GH],
                        qt[:, c, :],
                        start=(c == 0), stop=(c == G - 1),
                    )
                qp_ps[j, t] = ps

            def scores(j, t):
                """exp(scores.T) for s-tile t, shipped unnormalized bf16."""
                W = _width(t, causal)
                s0 = t * ST
                qi = s0 // SQ
                sq0 = s0 - qi * SQ

                # rope multiply reads qp straight from PSUM (f32), with a
                # stride-0 middle dim broadcasting qp over the {cos, tq} pair
                ps = qp_ps.pop((j, t))
                ab_sb = absb.tile([GH, 2, ST], FP16, name=f"ab_{j}_{t}",
                                  tag="ab")
                qp_b = bass.AP(
                    tensor=ps.tensor, offset=ps.offset,
                    ap=[ps.ap[0], [0, 2], ps.ap[1]],
                )
                if qi == 0:
                    ct_in = (ct0a_sb[:, :, :] if t == 0
                             else ct0b_sb[:, :, (t - 1) * ST:t * ST])
                else:
                    ct_in = ct_sb[qi][:, :, sq0:sq0 + ST]
                nc.vector.tensor_tensor(
                    out=ab_sb, in0=qp_b,
                    in1=ct_in,
                    op=mybir.AluOpType.mult,
                )
                # transposed score matmul: kpr/kq2 as weights, ab streams
                at_ps = atps.tile([NB, ST], F32, name=f"at_{j}_{t}", tag="at")
                nc.tensor.matmul(at_ps[:W, :], kpq_sb[:, j, 0, :W],
                                 ab_sb[:, 0, :], start=True, stop=False)
                nc.tensor.matmul(at_ps[:W, :], kpq_sb[:, j, 1, :W],
                                 ab_sb[:, 1, :], start=False, stop=True)
                o_sb = osb.tile([NB, ST], BF16, name=f"o_{j}_{t}", tag="o")
                nc.scalar.activation(o_sb[:W, :], at_ps[:W, :],
                                     mybir.ActivationFunctionType.Exp)
                nc.gpsimd.dma_start(out=out[j, t, :W, :], in_=o_sb[:W, :])

            # ---- emission: software-pipelined.  qproj for tile i+AHEAD is
            # emitted before scores(i) so the PE queue always has work while
            # the rope/score chain of tile i moves through the vector/scalar
            # engines; q DMAs are triggered LEAD tiles before their qproj ----
            AHEAD = 3
            LEAD = 4          # extra q-DMA trigger lead beyond AHEAD (tiles)
            for a in range(AHEAD):
                ja, ta = seq[a]
                load_q(ja, ta)
                qproj(ja, ta)
            for i, (j, t) in enumerate(seq):
                if i + AHEAD + LEAD < len(seq):
                    jb, tb = seq[i + AHEAD + LEAD]
                    load_q(jb, tb)
                    load_quarter(tb // TPC)
                if i + AHEAD < len(seq):
                    ja, ta = seq[i + AHEAD]
                    load_q(ja, ta)
                    load_quarter(ta // TPC)
                    qproj(ja, ta)
                scores(j, t)
    nc.compile()
    return nc


_BUILT = {}


def _get_built(causal):
    if causal not in _BUILT:
        _BUILT[causal] = build_bass(causal)
    return _BUILT[causal]


def _canonical_causal_mask():
    pos = np.arange(S)[:, None] // BLK
    blk = np.arange(NB)[None, :]
    return np.where(blk <= pos, 0.0, -1e9).astype(np.float32)


# Host-side plan shared between prep_inputs / run_cores / gather_output
# (kernel() calls them in sequence; test.py does the same).
_PLAN = {}


def prep_inputs(q, k, attention_mask, cos_q, sin_q, cos_k, sin_k, wq, wk):
    """Slice + lay out the full inputs into 8 per-core input maps (numpy)."""
    q = np.asarray(q, dtype=np.float32)
    k = np.asarray(k, dtype=np.float32)
    attention_mask = np.asarray(attention_mask, dtype=np.float32)
    cos_q = np.asarray(cos_q, dtype=np.float32)
    sin_q = np.asarray(sin_q, dtype=np.float32)
    cos_k = np.asarray(cos_k, dtype=np.float32)
    sin_k = np.asarray(sin_k, dtype=np.float32)
    wq = np.asarray(wq, dtype=np.float32)
    wk = np.asarray(wk, dtype=np.float32)

    causal = bool(
        attention_mask.shape == (B, 1, S, NB)
        and all(
            np.array_equal(attention_mask[b, 0], _canonical_causal_mask())
            for b in range(B)
        )
        and not os.environ.get("KERNEL_FORCE_GENERIC")
    )
    _PLAN.clear()
    _PLAN.update({"causal": causal, "mask": attention_mask})

    scale = np.float32(1.0 / math.sqrt(GH))
    wq_s = (wq * scale).astype(np.float32)                  # fold score scale

    # ---- host K path: block pooling, projection, rope (tiny: ~0.2% FLOPs)
    kb = k.reshape(B, NB, BLK, HK, D)
    kc = np.concatenate([kb.mean(axis=2), kb.max(axis=2)], axis=-1)
    # kp[b, n, h, o] = sum_i kc[b, n, h, i] * wk[h, i, o]
    kp = np.einsum('bnhi,hio->bnho', kc, wk)                # [B, NB, HK, GH]
    kp = kp.transpose(0, 2, 1, 3)                           # [B, HK, NB, GH]
    rot = np.concatenate([-kp[..., H:], kp[..., :H]], axis=-1)
    kpr = kp * cos_k[:, None] + rot * sin_k[:, None]        # [B, HK, NB, GH]
    kq2 = np.concatenate([kpr[..., H:], kpr[..., :H]], axis=-1)

    f16 = _np_dt(FP16)
    in_maps = []
    for core in range(N_CORES):
        b = core // 4
        kh0 = 2 * (core % 4)
        qs = q[b, :, 4 * kh0:4 * kh0 + 8, :]                # [S, 8, D]
        qTm = np.ascontiguousarray(
            qs.reshape(NT, ST, 2, G, D).transpose(2, 0, 4, 3, 1)  # [2,NT,D,G,ST]
        ).astype(f16)
        # tq[h, s] = sin[s, h+64] for h<64 ; -sin[s, h-64] for h>=64
        cq = cos_q[b]                                       # [S, GH]
        sq = sin_q[b]
        tqm = np.concatenate([sq[:, H:], -sq[:, :H]], axis=1).T
        ctm = np.ascontiguousarray(
            np.stack([cq.T, tqm], axis=1)                   # [GH, 2, S]
        ).astype(f16)
        # packed residents
        wqp = np.ascontiguousarray(
            wq_s[kh0:kh0 + 2].reshape(UNITS, G, D, GH)      # [2, G, D, GH]
            .transpose(2, 0, 1, 3).reshape(D, UNITS * G * GH)
        ).astype(f16)
        # kpq[gh, unit, {kpr, kq2}, nb]
        kpqm = np.ascontiguousarray(
            np.stack([kpr[b, kh0:kh0 + 2], kq2[b, kh0:kh0 + 2]], axis=1)
            .transpose(3, 0, 1, 2)                          # [GH, 2, 2, NB]
        ).astype(f16)
        in_maps.append({
            "qT": qTm,
            "wqp": wqp,
            "kpq": kpqm,
            "ct": ctm,
        })
    return in_maps


def run_cores(in_maps, **kwargs):
    nc = _get_built(_PLAN.get("causal", False))
    return run_bass_kernel_spmd(nc, in_maps, core_ids=list(range(N_CORES)),
                                **kwargs)


def gather_output(res):
    causal = _PLAN["causal"]
    mask = _PLAN["mask"]
    # device ships e = exp(scores).T unnormalized; reassemble [B, HK, S, NB]
    e = np.zeros((B, HK, S, NB), dtype=np.float32)
    for core in range(N_CORES):
        b = core // 4
        kh0 = 2 * (core % 4)
        o = np.asarray(res.results[core]["out"], dtype=np.float32)
        for j in range(UNITS):
            dst = e[b, kh0 + j].reshape(NT, ST, NB)
            for t in range(NT):
                W = _width(t, causal)
                dst[t, :, :W] = o[j, t, :W, :].T
    # host-side mask + normalization: softmax(x+m) = e^x e^(m-c) / sum,
    # with c = rowmax(m) (softmax is shift-invariant).  For the canonical
    # 0/-1e9 masks this is exactly e * {1, 0}.
    with np.errstate(under="ignore"):
        emask = np.exp(mask - mask.max(axis=-1, keepdims=True))
    w = e * emask                               # [B,1,S,NB] broadcasts over HK
    denom = w.sum(axis=-1, keepdims=True)
    bad = denom == 0.0
    if np.any(bad):
        # pathological rows (e underflow): recover logits and redo stably
        with np.errstate(divide="ignore"):
            x = np.log(np.maximum(e, 1e-38)) + mask
        x -= x.max(axis=-1, keepdims=True)
        w2 = np.exp(x)
        w = np.where(np.broadcast_to(bad, w.shape), w2, w)
        denom = w.sum(axis=-1, keepdims=True)
    return (w / denom).astype(np.float32)


def kernel(**inputs):
    in_maps = prep_inputs(**inputs)
    res = run_cores(in_maps)
    return gather_output(res)


# revision 36
# speedup vs baseline: 1.0155x; 1.0155x over previous
"""Trainium2 Bass kernel for nn_AttnGate (block-sparse attention gate).

Computation (per batch b, kv-head kh):
    qp  = einsum('s(gd),(gd)o->so', q[b,:,4kh:4kh+4,:], wq[kh])       # [S, GH]
    qpR = rope(qp, cos_q[b], sin_q[b])
    kc  = [mean, max] pooling of k[b,:,kh,:] over 64-key blocks        # [NB, 2D]
    kp  = kc @ wk[kh];  kpR = rope(kp, cos_k[b], sin_k[b])             # [NB, GH]
    out = softmax(qpR @ kpR.T / sqrt(GH) + mask[b])                    # [S, NB]

Sharding: 16 (b, kh) units over 8 cores; core c handles b = c//4 and
kh in {2*(c%4), 2*(c%4)+1}.  No collectives; host slices/gathers.

Device/host split: the device does the dominant 99.8% of FLOPs — the
q projection (1.07 GFLOP/unit), the q-side rope multiply, the QK score
matmul and the exp.  The tiny K path (block pooling + [NB,2D]@[2D,GH]
projection + rope, ~0.2% of FLOPs) runs on the host, which shrinks the
device input stream by the whole 4.2 MB k tensor; only the finished
kpr/kq2 tables (131 KB) ship.  As in the baseline, the additive mask
and the softmax normalization are applied on the host (softmax(x+m) =
e^x e^m / sum — identical math), so the device ships unnormalized
e = exp(scores) in bf16 and the mask never moves to the device.

Per-core HBM traffic: q 16.8 MB fp16 + rope tables 4.2 MB fp16 +
out ~2.2 MB bf16 (causal) + residents ~0.4 MB ~= 23.6 MB, streamed at
~24-26 GB/s/engine x 16 DMA engines (~410 GB/s sustained).  The kernel
is paced by this stream; compute per tile (PE 1.3 us, DVE 1.2 us,
scalar 0.7 us) sits under the ~1.7 us/tile DMA budget, so tiles drain
as they land.  DMA engines service descriptors in global enqueue
order, so input transfers trigger in consumption order with a ~7-tile
lead (a larger lead or out-of-order / front-loaded transfers displace
later-needed bytes and stall the pipeline -- measured, not
theoretical).  The first few transfers issue on the three
trigger-capable queues (sync, scalar, gpsimd) in parallel to beat the
~0.64 us/trigger single-queue rate during the ramp; the rest stays on
the sync queue.

Score matmul orientation: kpr/kq2 are the PE *weights* ([GH, W]) and
the roped q-side streams its full 512-column s-tile, producing the
score tile transposed ([W, 512] in PSUM).  This needs only 2 matmul
instructions per tile (vs 8 short W-moving ones), avoiding the ~150 ns
per-instruction PE overhead that dominated the old orientation; the
host gather untransposes.  RoPE rotate-half is folded into the QK
matmul: attn[n,s] = sum_h kpr[h,n]*(qp*cos)[h,s] + kq2[h,n]*(qp*tq)[h,s]
with tq = swap_halves(sin_q) (upper half negated) and kq2 =
swap_halves(kpR), both built on host.

When the mask is the canonical block-causal pattern, a "causal" NEFF
ships only the W = 8t+8 visible score rows for s-tile t (PE time is
W-independent in this orientation; only output bytes shrink).  Any
other mask falls back to a generic full-width NEFF.

Host-side layout prep (numpy only):
  - q pre-transposed to contraction-major tiles [D, G, ST] fp16
    (4 KB contiguous per partition row).
  - 1/sqrt(GH) folded into wq.
  - rope tables packed [GH, 2, S] fp16 = {cos_q, tq}, quartered.
"""

import math
import os
import sys

import numpy as np

for _p in ("/opt/trn_rl_repo", "/root/.axon_site/_ro/trn_rl_repo"):
    if os.path.isdir(_p) and _p not in sys.path:
        sys.path.append(_p)

import ml_dtypes  # noqa: E402,F401

import concourse.bass as bass  # noqa: E402,F401
import concourse.bacc as bacc  # noqa: E402
import concourse.mybir as mybir  # noqa: E402
from concourse.bass_utils import run_bass_kernel_spmd  # noqa: E402
from concourse.tile import TileContext  # noqa: E402

# Problem dims (hardcoded per spec).
B, S, HQ, HK, D, GH = 2, 8192, 32, 8, 128, 128
BLK = 64
NB = S // BLK          # 128 key blocks
G = HQ // HK           # 4
GD = G * D             # 512 contraction for the q projection
H = GH // 2            # rotate-half split
ST = 512               # s-tile (matmul moving-dim = PSUM bank)
NT = S // ST           # 16 s-tiles per unit
TPC = 4                # s-tiles per q DMA chunk
NCH = NT // TPC        # 4 q chunks per unit
N_CORES = 8
UNITS = 2              # (b, kh) units per core

F32 = mybir.dt.float32
BF16 = mybir.dt.bfloat16
FP16 = mybir.dt.float16
FP = mybir.dt.np  # dt -> numpy dtype


def _np_dt(dt):
    return np.dtype(FP(dt))


def _width(t, causal):
    """Score rows computed for s-tile `t` (all unless causal)."""
    return 8 * t + 8 if causal else NB


def build_bass(causal):
    """Build the single-core SPMD Bass program (same NEFF on all 8 cores)."""
    nc = bacc.Bacc("TRN2")

    qT = nc.declare_dram_parameter("qT", [UNITS, NT, D, G, ST], FP16,
                                   isOutput=False)
    # packed small residents: wq [D, UNITS*G*GH] fp16, kpq [GH, UNITS, 2, NB]
    wqp = nc.declare_dram_parameter("wqp", [D, UNITS * G * GH], FP16,
                                    isOutput=False)
    kpq = nc.declare_dram_parameter("kpq", [GH, UNITS, 2, NB], FP16,
                                    isOutput=False)
    # q-side rope tables {cos, tq}, streamed in quarters
    ct = nc.declare_dram_parameter("ct", [GH, 2, S], FP16, isOutput=False)
    out = nc.declare_dram_parameter("out", [UNITS, NT, NB, ST], BF16,
                                    isOutput=True)

    NQUART = 4
    SQ = S // NQUART  # 2048 cols per rope-table quarter

    seq = [(j, t) for j in range(UNITS) for t in range(NT)]

    with TileContext(nc) as tc:
        with (
            tc.tile_pool(name="singles", bufs=1) as singles,
            tc.tile_pool(name="qin", bufs=10) as qin,
            tc.tile_pool(name="qps", bufs=4, space="PSUM") as qps,
            tc.tile_pool(name="atps", bufs=4, space="PSUM") as atps,
            tc.tile_pool(name="absb", bufs=6) as absb,
            tc.tile_pool(name="osb", bufs=6) as osb,
        ):
            # ---- resident + streamed DMAs, ordered so the first qproj is
            # gated on only 0.76 MB (wq unit-0 half + q tile 0) and the first
            # score chain on ~1.2 MB, instead of the whole resident set ----
            # The sync queue can only issue one DMA trigger per ~0.64 us, so
            # a single-queue start leaves the 16 DMA engines starved for the
            # first ~4 us.  The first five transfers therefore trigger on
            # the three trigger-capable queues (sync, scalar, gpsimd) in
            # parallel; the rest of the stream stays on sync in order.
            wq_sb = {}
            for j in range(UNITS):
                wq_sb[j] = singles.tile([D, G * GH], FP16, name=f"wq_sb{j}",
                                        tag=f"wq{j}")
            nc.sync.dma_start(out=wq_sb[0], in_=wqp[:, :G * GH])

            q_tiles = {}

            def load_q(j, t, eng=None):
                if (j, t) not in q_tiles:
                    tl = qin.tile([D, G, ST], FP16, name=f"q_{j}_{t}", tag="q")
                    (eng or nc.sync).dma_start(out=tl, in_=qT[j, t])
                    q_tiles[j, t] = tl

            load_q(0, 0, eng=nc.scalar)

            # ct quarter 0 ships split (tile-0 slice + tiles 1-3) so rope(0)
            # is not gated behind the full 1 MB quarter during the DMA ramp
            ct0a_sb = singles.tile([GH, 2, ST], FP16, name="ct0a_sb",
                                   tag="ct0a")
            nc.gpsimd.dma_start(out=ct0a_sb, in_=ct[:, :, :ST])
            kpq_sb = singles.tile([GH, UNITS, 2, NB], FP16, name="kpq_sb",
                                  tag="kpq")
            nc.sync.dma_start(out=kpq_sb, in_=kpq[:, :, :, :])
            load_q(0, 1, eng=nc.scalar)

            ct_sb = {}

            def load_quarter(qi):
                if 0 < qi < NQUART and qi not in ct_sb:
                    tl = singles.tile([GH, 2, SQ], FP16, name=f"ct_sb_{qi}",
                                      tag=f"ctq{qi}")
                    nc.sync.dma_start(out=tl,
                                      in_=ct[:, :, qi * SQ:(qi + 1) * SQ])
                    ct_sb[qi] = tl

            load_q(0, 2)
            ct0b_sb = singles.tile([GH, 2, SQ - ST], FP16, name="ct0b_sb",
                                   tag="ct0b")
            nc.sync.dma_start(out=ct0b_sb, in_=ct[:, :, ST:SQ])
            load_q(0, 3)
            nc.sync.dma_start(out=wq_sb[1], in_=wqp[:, G * GH:])
            load_quarter(1)

            # ---- compute building blocks ----
            qp_ps = {}

            def qproj(j, t):
                ps = qps.tile([GH, ST], F32, name=f"qp_{j}_{t}", tag="qp")
                qt = q_tiles[j, t]
                for c in range(G):
                    nc.tensor.matmul(
                        ps, wq_sb[j][:, c * GH:(c + 1) * GH],
                        qt[:, c, :],
                        start=(c == 0), stop=(c == G - 1),
                    )
                qp_ps[j, t] = ps

            def scores(j, t):
                """exp(scores.T) for s-tile t, shipped unnormalized bf16."""
                W = _width(t, causal)
                s0 = t * ST
                qi = s0 // SQ
                sq0 = s0 - qi * SQ

                # rope multiply reads qp straight from PSUM (f32), with a
                # stride-0 middle dim broadcasting qp over the {cos, tq} pair
                ps = qp_ps.pop((j, t))
                ab_sb = absb.tile([GH, 2, ST], FP16, name=f"ab_{j}_{t}",
                                  tag="ab")
                qp_b = bass.AP(
                    tensor=ps.tensor, offset=ps.offset,
                    ap=[ps.ap[0], [0, 2], ps.ap[1]],
                )
                if qi == 0:
                    ct_in = (ct0a_sb[:, :, :] if t == 0
                             else ct0b_sb[:, :, (t - 1) * ST:t * ST])
                else:
                    ct_in = ct_sb[qi][:, :, sq0:sq0 + ST]
                nc.vector.tensor_tensor(
                    out=ab_sb, in0=qp_b,
                    in1=ct_in,
                    op=mybir.AluOpType.mult,
                )
                # transposed score matmul: kpr/kq2 as weights, ab streams
                at_ps = atps.tile([NB, ST], F32, name=f"at_{j}_{t}", tag="at")
                nc.tensor.matmul(at_ps[:W, :], kpq_sb[:, j, 0, :W],
                                 ab_sb[:, 0, :], start=True, stop=False)
                nc.tensor.matmul(at_ps[:W, :], kpq_sb[:, j, 1, :W],
                                 ab_sb[:, 1, :], start=False, stop=True)
                o_sb = osb.tile([NB, ST], BF16, name=f"o_{j}_{t}", tag="o")
                nc.scalar.activation(o_sb[:W, :], at_ps[:W, :],
                                     mybir.ActivationFunctionType.Exp)
                nc.gpsimd.dma_start(out=out[j, t, :W, :], in_=o_sb[:W, :])

            # ---- emission: software-pipelined.  qproj for tile i+AHEAD is
            # emitted before scores(i) so the PE queue always has work while
            # the rope/score chain of tile i moves through the vector/scalar
            # engines; q DMAs are triggered LEAD tiles before their qproj ----
            AHEAD = 3
            LEAD = 4          # extra q-DMA trigger lead beyond AHEAD (tiles)
            for a in range(AHEAD):
                ja, ta = seq[a]
                load_q(ja, ta)
                qproj(ja, ta)
            for i, (j, t) in enumerate(seq):
                if i + AHEAD + LEAD < len(seq):
                    jb, tb = seq[i + AHEAD + LEAD]
                    load_q(jb, tb)
                    load_quarter(tb // TPC)
                if i + AHEAD < len(seq):
                    ja, ta = seq[i + AHEAD]
                    load_q(ja, ta)
                    load_quarter(ta // TPC)
                    qproj(ja, ta)
                scores(j, t)
    nc.compile()
    return nc


_BUILT = {}


def _get_built(causal):
    if causal not in _BUILT:
        _BUILT[causal] = build_bass(causal)
    return _BUILT[causal]


def _canonical_causal_mask():
    pos = np.arange(S)[:, None] // BLK
    blk = np.arange(NB)[None, :]
    return np.where(blk <= pos, 0.0, -1e9).astype(np.float32)


# Host-side plan shared between prep_inputs / run_cores / gather_output
# (kernel() calls them in sequence; test.py does the same).
_PLAN = {}


def prep_inputs(q, k, attention_mask, cos_q, sin_q, cos_k, sin_k, wq, wk):
    """Slice + lay out the full inputs into 8 per-core input maps (numpy)."""
    q = np.asarray(q, dtype=np.float32)
    k = np.asarray(k, dtype=np.float32)
    attention_mask = np.asarray(attention_mask, dtype=np.float32)
    cos_q = np.asarray(cos_q, dtype=np.float32)
    sin_q = np.asarray(sin_q, dtype=np.float32)
    cos_k = np.asarray(cos_k, dtype=np.float32)
    sin_k = np.asarray(sin_k, dtype=np.float32)
    wq = np.asarray(wq, dtype=np.float32)
    wk = np.asarray(wk, dtype=np.float32)

    causal = bool(
        attention_mask.shape == (B, 1, S, NB)
        and all(
            np.array_equal(attention_mask[b, 0], _canonical_causal_mask())
            for b in range(B)
        )
        and not os.environ.get("KERNEL_FORCE_GENERIC")
    )
    _PLAN.clear()
    _PLAN.update({"causal": causal, "mask": attention_mask})

    scale = np.float32(1.0 / math.sqrt(GH))
    wq_s = (wq * scale).astype(np.float32)                  # fold score scale

    # ---- host K path: block pooling, projection, rope (tiny: ~0.2% FLOPs)
    kb = k.reshape(B, NB, BLK, HK, D)
    kc = np.concatenate([kb.mean(axis=2), kb.max(axis=2)], axis=-1)
    # kp[b, n, h, o] = sum_i kc[b, n, h, i] * wk[h, i, o]
    kp = np.einsum('bnhi,hio->bnho', kc, wk)                # [B, NB, HK, GH]
    kp = kp.transpose(0, 2, 1, 3)                           # [B, HK, NB, GH]
    rot = np.concatenate([-kp[..., H:], kp[..., :H]], axis=-1)
    kpr = kp * cos_k[:, None] + rot * sin_k[:, None]        # [B, HK, NB, GH]
    kq2 = np.concatenate([kpr[..., H:], kpr[..., :H]], axis=-1)

    f16 = _np_dt(FP16)
    in_maps = []
    for core in range(N_CORES):
        b = core // 4
        kh0 = 2 * (core % 4)
        qs = q[b, :, 4 * kh0:4 * kh0 + 8, :]                # [S, 8, D]
        qTm = np.ascontiguousarray(
            qs.reshape(NT, ST, 2, G, D).transpose(2, 0, 4, 3, 1)  # [2,NT,D,G,ST]
        ).astype(f16)
        # tq[h, s] = sin[s, h+64] for h<64 ; -sin[s, h-64] for h>=64
        cq = cos_q[b]                                       # [S, GH]
        sq = sin_q[b]
        tqm = np.concatenate([sq[:, H:], -sq[:, :H]], axis=1).T
        ctm = np.ascontiguousarray(
            np.stack([cq.T, tqm], axis=1)                   # [GH, 2, S]
        ).astype(f16)
        # packed residents
        wqp = np.ascontiguousarray(
            wq_s[kh0:kh0 + 2].reshape(UNITS, G, D, GH)      # [2, G, D, GH]
            .transpose(2, 0, 1, 3).reshape(D, UNITS * G * GH)
        ).astype(f16)
        # kpq[gh, unit, {kpr, kq2}, nb]
        kpqm = np.ascontiguousarray(
            np.stack([kpr[b, kh0:kh0 + 2], kq2[b, kh0:kh0 + 2]], axis=1)
            .transpose(3, 0, 1, 2)                          # [GH, 2, 2, NB]
        ).astype(f16)
        in_maps.append({
            "qT": qTm,
            "wqp": wqp,
            "kpq": kpqm,
            "ct": ctm,
        })
    return in_maps


def run_cores(in_maps, **kwargs):
    nc = _get_built(_PLAN.get("causal", False))
    return run_bass_kernel_spmd(nc, in_maps, core_ids=list(range(N_CORES)),
                                **kwargs)


def gather_output(res):
    causal = _PLAN["causal"]
    mask = _PLAN["mask"]
    # device ships e = exp(scores).T unnormalized; reassemble [B, HK, S, NB]
    e = np.zeros((B, HK, S, NB), dtype=np.float32)
    for core in range(N_CORES):
        b = core // 4
        kh0 = 2 * (core % 4)
        o = np.asarray(res.results[core]["out"], dtype=np.float32)
        for j in range(UNITS):
            dst = e[b, kh0 + j].reshape(NT, ST, NB)
            for t in range(NT):
                W = _width(t, causal)
                dst[t, :, :W] = o[j, t, :W, :].T
    # host-side mask + normalization: softmax(x+m) = e^x e^(m-c) / sum,
    # with c = rowmax(m) (softmax is shift-invariant).  For the canonical
    # 0/-1e9 masks this is exactly e * {1, 0}.
    with np.errstate(under="ignore"):
        emask = np.exp(mask - mask.max(axis=-1, keepdims=True))
    w = e * emask                               # [B,1,S,NB] broadcasts over HK
    denom = w.sum(axis=-1, keepdims=True)
    bad = denom == 0.0
    if np.any(bad):
        # pathological rows (e underflow): recover logits and redo stably
        with np.errstate(divide="ignore"):
            x = np.log(np.maximum(e, 1e-38)) + mask
        x -= x.max(axis=-1, keepdims=True)
        w2 = np.exp(x)
        w = np.where(np.broadcast_to(bad, w.shape), w2, w)
        denom = w.sum(axis=-1, keepdims=True)
    return (w / denom).astype(np.float32)


def kernel(**inputs):
    in_maps = prep_inputs(**inputs)
    res = run_cores(in_maps)
    return gather_output(res)
